# Initial kernel scaffold
#
import sys

if "/opt/trn_rl_repo" not in sys.path:
    sys.path.insert(0, "/opt/trn_rl_repo")

import numpy as np
import ml_dtypes

import concourse.bass as bass
import concourse.tile as tile
from concourse import bacc
from concourse.bass_utils import run_bass_kernel_spmd
from concourse.masks import make_identity

mybir = bass.mybir

N = 8192
M = 8192
D = 256
H = 128
NCORES = 8
RPC = N // NCORES  # 1024 rows per core
MPC = M // NCORES  # 1024 hyperlink cols per core (RS split)

BF16 = ml_dtypes.bfloat16

PROFILE = False
LAST_EXEC_NS = None
LAST_TRACE_DIR = None

_NC = None


def _emit(nc, tc, ins, out_ap):
    dt = mybir.dt
    Alu = mybir.AluOpType
    Act = mybir.ActivationFunctionType

    with (
        tc.tile_pool(name="persist", bufs=1) as pp,
        tc.tile_pool(name="dram", bufs=1, space="DRAM") as dram,
    ):
        # ---- persistent SBUF tensors ----
        gcnwt_s = pp.tile([H, D], dt.bfloat16)
        loopwt_s = pp.tile([H, D], dt.bfloat16)
        intw_s = pp.tile([H, 1], dt.bfloat16)
        gcnb2_s = pp.tile([H, 2], dt.bfloat16)
        loopb2_s = pp.tile([H, 2], dt.bfloat16)
        intb2_s = pp.tile([2, 1], dt.float32)
        ixt0_s = pp.tile([128, RPC], dt.bfloat16)
        ixt1_s = pp.tile([128, RPC], dt.bfloat16)
        jxt0_s = pp.tile([128, RPC], dt.bfloat16)
        jxt1_s = pp.tile([128, RPC], dt.bfloat16)
        ones_row = pp.tile([1, 128], dt.bfloat16)
        ident2 = pp.tile([2, 2], dt.bfloat16)
        vg = pp.tile([128, 2], dt.bfloat16)
        lv = pp.tile([128, 2], dt.bfloat16)
        c2 = pp.tile([1, 2], dt.bfloat16)
        d3 = pp.tile([1, 3], dt.bfloat16)
        p_ext = pp.tile([128, M // 128, 2], dt.bfloat16)
        wT_s = pp.tile([2, RPC], dt.bfloat16)
        w_ext = pp.tile([128, RPC // 128, 3], dt.bfloat16)

        nc.sync.dma_start(gcnwt_s[:], ins["gcnwt"][:])
        nc.sync.dma_start(loopwt_s[:], ins["loopwt"][:])
        nc.sync.dma_start(intw_s[:], ins["intw"][:])
        nc.sync.dma_start(gcnb2_s[:], ins["gcnb2"][:])
        nc.sync.dma_start(loopb2_s[:], ins["loopb2"][:])
        nc.sync.dma_start(intb2_s[:], ins["intb2"][:])
        nc.sync.dma_start(ixt0_s[:], ins["ixt"][0:128, :])
        nc.sync.dma_start(ixt1_s[:], ins["ixt"][128:256, :])
        nc.sync.dma_start(jxt0_s[:], ins["jxt"][0:128, :])
        nc.sync.dma_start(jxt1_s[:], ins["jxt"][128:256, :])
        nc.vector.memset(ones_row[:], 1.0)
        make_identity(nc, ident2[:])

        # ---- stage A: fold int_w through weights ----
        with tc.tile_pool(name="aps", bufs=2, space=bass.MemorySpace.PSUM) as aps_pool:
            for wt_s, dst in ((gcnwt_s, vg), (loopwt_s, lv)):
                ps = aps_pool.tile([128, 2], dt.float32)
                for ch in range(2):
                    nc.tensor.matmul(
                        ps[:, ch : ch + 1],
                        wt_s[:, ch * 128 : (ch + 1) * 128],
                        intw_s[:],
                        start=True,
                        stop=True,
                    )
                nc.vector.tensor_copy(dst[:], ps[:])
            for b_s, dst in ((gcnb2_s, c2), (loopb2_s, d3)):
                ps = aps_pool.tile([1, 2], dt.float32)
                nc.tensor.matmul(ps[:], intw_s[:], b_s[:], start=True, stop=True)
                nc.vector.tensor_copy(dst[:, 0:2], ps[:])
        nc.vector.memset(d3[:, 2:3], 1.0)

        # ---- stage B: p = AX @ v + c, laid out [128, 64, 2] (m on partitions) ----
        with (
            tc.tile_pool(name="ax", bufs=2) as ax_pool,
            tc.tile_pool(name="bps", bufs=2, space=bass.MemorySpace.PSUM) as bps_pool,
        ):
            for mg in range(M // 512):
                axi0 = ax_pool.tile([128, 512], dt.bfloat16)
                axi1 = ax_pool.tile([128, 512], dt.bfloat16)
                axj0 = ax_pool.tile([128, 512], dt.bfloat16)
                axj1 = ax_pool.tile([128, 512], dt.bfloat16)
                sl = slice(mg * 512, (mg + 1) * 512)
                nc.sync.dma_start(axi0[:], ins["iaxt"][0:128, sl])
                nc.sync.dma_start(axi1[:], ins["iaxt"][128:256, sl])
                nc.sync.dma_start(axj0[:], ins["jaxt"][0:128, sl])
                nc.sync.dma_start(axj1[:], ins["jaxt"][128:256, sl])
                for mt in range(4):
                    mc = mg * 4 + mt
                    msl = slice(mt * 128, (mt + 1) * 128)
                    psp = bps_pool.tile([128, 2], dt.float32)
                    nc.tensor.matmul(
                        psp[:, 0:2], ones_row[:], c2[:], start=True, stop=False
                    )
                    for src, col in ((axi0, 0), (axi1, 0), (axj0, 1), (axj1, 1)):
                        vch = 0 if src in (axi0, axj0) else 1
                        nc.tensor.matmul(
                            psp[:, col : col + 1],
                            src[:, msl],
                            vg[:, vch : vch + 1],
                            start=False,
                            stop=(src is axj1),
                            skip_group_check=True,
                        )
                    nc.vector.tensor_copy(p_ext[:, mc, :], psp[:])

        # ---- stage C: pass 1, wT[2, RPC] = p.T @ I[rows].T ----
        with (
            tc.tile_pool(name="it", bufs=3) as it_pool,
            tc.tile_pool(name="cps", bufs=2, space=bass.MemorySpace.PSUM) as cps_pool,
        ):
            for nh in range(RPC // 512):
                wps = cps_pool.tile([2, 512], dt.float32)
                for mc in range(M // 128):
                    it = it_pool.tile([128, 512], dt.bfloat16)
                    nc.sync.dma_start(
                        it[:],
                        ins["ist"][
                            mc * 128 : (mc + 1) * 128, nh * 512 : (nh + 1) * 512
                        ],
                    )
                    nc.tensor.matmul(
                        wps[:],
                        p_ext[:, mc, :],
                        it[:],
                        start=(mc == 0),
                        stop=(mc == M // 128 - 1),
                    )
                nc.vector.tensor_copy(wT_s[:, nh * 512 : (nh + 1) * 512], wps[:])

        # ---- stage D: assemble w_ext[n, nt, (w_i, w_j, 1)] ----
        with tc.tile_pool(name="dps", bufs=2, space=bass.MemorySpace.PSUM) as dps_pool:
            for nt in range(RPC // 128):
                nsl = slice(nt * 128, (nt + 1) * 128)
                aps = dps_pool.tile([128, 3], dt.float32)
                nc.tensor.matmul(
                    aps[:, 0:3], ones_row[:], d3[:], start=True, stop=False
                )
                nc.tensor.matmul(
                    aps[:, 0:2],
                    wT_s[0:2, nsl],
                    ident2[:],
                    start=False,
                    stop=False,
                    skip_group_check=True,
                )
                for x_s, col, vch in (
                    (ixt0_s, 0, 0),
                    (ixt1_s, 0, 1),
                    (jxt0_s, 1, 0),
                    (jxt1_s, 1, 1),
                ):
                    nc.tensor.matmul(
                        aps[:, col : col + 1],
                        x_s[:, nsl],
                        lv[:, vch : vch + 1],
                        start=False,
                        stop=(x_s is jxt1_s),
                        skip_group_check=True,
                    )
                nc.vector.tensor_copy(w_ext[:, nt, :], aps[:])

        # ---- stage E: pass 2, U[3, M] = w_ext.T @ I[rows], blocked for RS ----
        u_d = dram.tile([NCORES, 3, MPC], dt.float32)
        r_d = dram.tile([3, MPC], dt.float32)
        with (
            tc.tile_pool(name="it2", bufs=3) as it2_pool,
            tc.tile_pool(name="eps", bufs=2, space=bass.MemorySpace.PSUM) as eps_pool,
            tc.tile_pool(name="ue", bufs=2) as ue_pool,
        ):
            for mb in range(NCORES):
                for mh in range(MPC // 512):
                    base = mb * MPC + mh * 512
                    ups = eps_pool.tile([3, 512], dt.float32)
                    for nch in range(RPC // 128):
                        it2 = it2_pool.tile([128, 512], dt.bfloat16)
                        nc.sync.dma_start(
                            it2[:],
                            ins["inat"][
                                nch * 128 : (nch + 1) * 128, base : base + 512
                            ],
                        )
                        nc.tensor.matmul(
                            ups[:],
                            w_ext[:, nch, :],
                            it2[:],
                            start=(nch == 0),
                            stop=(nch == RPC // 128 - 1),
                        )
                    ue = ue_pool.tile([3, 512], dt.float32)
                    nc.vector.tensor_copy(ue[:], ups[:])
                    nc.sync.dma_start(
                        u_d[mb, 0:3, mh * 512 : (mh + 1) * 512], ue[:]
                    )

        # ---- stage F: ReduceScatter over hyperlink blocks ----
        nc.gpsimd.collective_compute(
            "ReduceScatter",
            Alu.add,
            replica_groups=[list(range(NCORES))],
            ins=[u_d.ap().opt()],
            outs=[r_d.ap().opt()],
        )

        # ---- stage G: epilogue ----
        with tc.tile_pool(name="epi", bufs=1) as epi_pool:
            rs_s = epi_pool.tile([3, MPC], dt.float32)
            nc.sync.dma_start(rs_s[:], r_d[:])
            rec = epi_pool.tile([1, MPC], dt.float32)
            nc.vector.reciprocal(rec[:], rs_s[2:3, :])
            q = epi_pool.tile([2, MPC], dt.float32)
            for ch in range(2):
                nc.vector.scalar_tensor_tensor(
                    q[ch : ch + 1, :],
                    rs_s[ch : ch + 1, :],
                    1.0,
                    rec[:],
                    op0=Alu.mult,
                    op1=Alu.mult,
                )
            s_s = epi_pool.tile([2, MPC], dt.float32)
            nc.scalar.activation(s_s[:], q[:], Act.Sigmoid, bias=intb2_s[:])
            nc.sync.dma_start(out_ap[:], s_s[:])


def _build():
    global _NC
    if _NC is not None:
        return _NC
    nc = bacc.Bacc(
        "TRN2",
        target_bir_lowering=False,
        debug=False,
        enable_asserts=True,
        num_devices=NCORES,
    )
    dt = mybir.dt
    specs = [
        ("ist", [M, RPC], dt.bfloat16),
        ("inat", [RPC, M], dt.bfloat16),
        ("iaxt", [D, M], dt.bfloat16),
        ("jaxt", [D, M], dt.bfloat16),
        ("ixt", [D, RPC], dt.bfloat16),
        ("jxt", [D, RPC], dt.bfloat16),
        ("gcnwt", [H, D], dt.bfloat16),
        ("loopwt", [H, D], dt.bfloat16),
        ("intw", [H, 1], dt.bfloat16),
        ("gcnb2", [H, 2], dt.bfloat16),
        ("loopb2", [H, 2], dt.bfloat16),
        ("intb2", [2, 1], dt.float32),
    ]
    ins = {
        name: nc.dram_tensor(name, shape, d, kind="ExternalInput").ap()
        for name, shape, d in specs
    }
    out_ap = nc.dram_tensor("out", [2, MPC], dt.float32, kind="ExternalOutput").ap()
    with tile.TileContext(nc) as tc:
        _emit(nc, tc, ins, out_ap)
    nc.compile()
    _NC = nc
    return nc


def kernel(iX, jX, iAX, jAX, I, loop_w, loop_b, gcn_w, gcn_b, int_w, int_b):
    global LAST_EXEC_NS, LAST_TRACE_DIR
    nc = _build()

    iaxt = iAX.T.astype(BF16)
    jaxt = jAX.T.astype(BF16)
    gcnwt = gcn_w.T.astype(BF16)
    loopwt = loop_w.T.astype(BF16)
    intw = int_w.astype(BF16)
    gcnb2 = np.repeat(gcn_b.reshape(H, 1), 2, axis=1).astype(BF16)
    loopb2 = np.repeat(loop_b.reshape(H, 1), 2, axis=1).astype(BF16)
    intb2 = np.full((2, 1), np.float32(int_b.reshape(-1)[0]), dtype=np.float32)

    in_maps = []
    for k in range(NCORES):
        rows = slice(k * RPC, (k + 1) * RPC)
        in_maps.append(
            {
                "ist": I[rows].T.astype(BF16),
                "inat": I[rows].astype(BF16),
                "iaxt": iaxt,
                "jaxt": jaxt,
                "ixt": iX[rows].T.astype(BF16),
                "jxt": jX[rows].T.astype(BF16),
                "gcnwt": gcnwt,
                "loopwt": loopwt,
                "intw": intw,
                "gcnb2": gcnb2,
                "loopb2": loopb2,
                "intb2": intb2,
            }
        )

    res = run_bass_kernel_spmd(
        nc, in_maps, core_ids=list(range(NCORES)), trace=PROFILE
    )
    LAST_EXEC_NS = getattr(res, "exec_time_ns", None)
    LAST_TRACE_DIR = getattr(res, "tmpdir", None)

    S = np.empty((M, 1), dtype=np.float32)
    S_ = np.empty((M, 1), dtype=np.float32)
    for k in range(NCORES):
        o = np.asarray(res.results[k]["out"], dtype=np.float32)
        S[k * MPC : (k + 1) * MPC, 0] = o[0]
        S_[k * MPC : (k + 1) * MPC, 0] = o[1]
    return (S, S_)


# revision 13
# speedup vs baseline: 1.0727x; 1.0727x over previous
import sys

if "/opt/trn_rl_repo" not in sys.path:
    sys.path.insert(0, "/opt/trn_rl_repo")

import numpy as np
import ml_dtypes

import concourse.bass as bass
import concourse.tile as tile
from concourse import bacc
from concourse.bass_utils import run_bass_kernel_spmd

mybir = bass.mybir

N = 8192
M = 8192
D = 256
H = 128
NCORES = 8
RPC = N // NCORES  # 1024 rows per core
MPC = M // NCORES  # 1024 hyperlink cols per core (RS split)

BF16 = ml_dtypes.bfloat16

PROFILE = False
LAST_EXEC_NS = None
LAST_TRACE_DIR = None

_NC = None


def _emit(nc, tc, ins, out_ap):
    dt = mybir.dt
    Alu = mybir.AluOpType
    Act = mybir.ActivationFunctionType

    with (
        tc.tile_pool(name="persist", bufs=1) as pp,
        tc.tile_pool(name="dram", bufs=1, space="DRAM") as dram,
    ):
        # ---- persistent SBUF tensors ----
        # The p-chain (c, vg, iAX products) must be f32: q = I.T@(I@p)/colsum
        # amplifies any bias in mean(p) by ~4096x, so bf16 rounding there
        # shifts the logits by O(0.3). p is then split into a bf16 hi+lo
        # pair so the big pass-1 matmul can stay bf16.
        gcnwt_s = pp.tile([H, D], dt.float32)
        loopwt_s = pp.tile([H, D], dt.float32)
        intw_s = pp.tile([H, 1], dt.float32)
        gcnb2_s = pp.tile([H, 2], dt.float32)
        loopb2_s = pp.tile([H, 2], dt.float32)
        intb2_s = pp.tile([2, 1], dt.float32)
        ixt0_s = pp.tile([128, RPC], dt.bfloat16)
        ixt1_s = pp.tile([128, RPC], dt.bfloat16)
        jxt0_s = pp.tile([128, RPC], dt.bfloat16)
        jxt1_s = pp.tile([128, RPC], dt.bfloat16)
        ones_row = pp.tile([1, 128], dt.bfloat16)
        ones_f32 = pp.tile([1, 128], dt.float32)
        merge4_s = pp.tile([4, 2], dt.bfloat16)
        vg = pp.tile([128, 2], dt.float32)
        lv = pp.tile([128, 2], dt.bfloat16)
        c2 = pp.tile([1, 2], dt.float32)
        d3 = pp.tile([1, 3], dt.bfloat16)
        p_ext = pp.tile([128, M // 128, 4], dt.bfloat16)
        wT_s = pp.tile([4, RPC], dt.bfloat16)
        w_ext = pp.tile([128, RPC // 128, 3], dt.bfloat16)

        nc.sync.dma_start(gcnwt_s[:], ins["gcnwt"][:])
        nc.sync.dma_start(loopwt_s[:], ins["loopwt"][:])
        nc.sync.dma_start(intw_s[:], ins["intw"][:])
        nc.sync.dma_start(gcnb2_s[:], ins["gcnb2"][:])
        nc.sync.dma_start(loopb2_s[:], ins["loopb2"][:])
        nc.sync.dma_start(intb2_s[:], ins["intb2"][:])
        nc.sync.dma_start(merge4_s[:], ins["merge4"][:])
        nc.sync.dma_start(ixt0_s[:], ins["ixt"][0:128, :])
        nc.sync.dma_start(ixt1_s[:], ins["ixt"][128:256, :])
        nc.sync.dma_start(jxt0_s[:], ins["jxt"][0:128, :])
        nc.sync.dma_start(jxt1_s[:], ins["jxt"][128:256, :])
        nc.vector.memset(ones_row[:], 1.0)
        nc.vector.memset(ones_f32[:], 1.0)

        # ---- stage A: fold int_w through weights (all f32) ----
        with tc.tile_pool(name="aps", bufs=2, space=bass.MemorySpace.PSUM) as aps_pool:
            for wt_s, dst in ((gcnwt_s, vg), (loopwt_s, lv)):
                ps = aps_pool.tile([128, 2], dt.float32)
                for ch in range(2):
                    nc.tensor.matmul(
                        ps[:, ch : ch + 1],
                        wt_s[:, ch * 128 : (ch + 1) * 128],
                        intw_s[:],
                        start=True,
                        stop=True,
                    )
                nc.vector.tensor_copy(dst[:], ps[:])
            for b_s, dst in ((gcnb2_s, c2), (loopb2_s, d3)):
                ps = aps_pool.tile([1, 2], dt.float32)
                nc.tensor.matmul(ps[:], intw_s[:], b_s[:], start=True, stop=True)
                nc.vector.tensor_copy(dst[:, 0:2], ps[:])
        nc.vector.memset(d3[:, 2:3], 1.0)

        # ---- stage B: p = AX @ v + c in f32, split to bf16 hi/lo pair,
        # laid out [128, 64, (i_hi, j_hi, i_lo, j_lo)] (m on partitions) ----
        with (
            tc.tile_pool(name="ax", bufs=2) as ax_pool,
            tc.tile_pool(name="spl", bufs=2) as spl_pool,
            tc.tile_pool(name="bps", bufs=2, space=bass.MemorySpace.PSUM) as bps_pool,
        ):
            for mg in range(M // 512):
                axi0 = ax_pool.tile([128, 512], dt.float32)
                axi1 = ax_pool.tile([128, 512], dt.float32)
                axj0 = ax_pool.tile([128, 512], dt.float32)
                axj1 = ax_pool.tile([128, 512], dt.float32)
                sl = slice(mg * 512, (mg + 1) * 512)
                nc.sync.dma_start(axi0[:], ins["iaxt"][0:128, sl])
                nc.sync.dma_start(axi1[:], ins["iaxt"][128:256, sl])
                nc.sync.dma_start(axj0[:], ins["jaxt"][0:128, sl])
                nc.sync.dma_start(axj1[:], ins["jaxt"][128:256, sl])
                for mt in range(4):
                    mc = mg * 4 + mt
                    msl = slice(mt * 128, (mt + 1) * 128)
                    psp = bps_pool.tile([128, 2], dt.float32)
                    nc.tensor.matmul(
                        psp[:, 0:2], ones_f32[:], c2[:], start=True, stop=False
                    )
                    for src, col in ((axi0, 0), (axi1, 0), (axj0, 1), (axj1, 1)):
                        vch = 0 if src in (axi0, axj0) else 1
                        nc.tensor.matmul(
                            psp[:, col : col + 1],
                            src[:, msl],
                            vg[:, vch : vch + 1],
                            start=False,
                            stop=(src is axj1),
                            skip_group_check=True,
                        )
                    nc.vector.tensor_copy(p_ext[:, mc, 0:2], psp[:])
                    hi_f32 = spl_pool.tile([128, 2], dt.float32)
                    nc.vector.tensor_copy(hi_f32[:], p_ext[:, mc, 0:2])
                    nc.vector.scalar_tensor_tensor(
                        p_ext[:, mc, 2:4],
                        psp[:],
                        1.0,
                        hi_f32[:],
                        op0=Alu.mult,
                        op1=Alu.subtract,
                    )

        # ---- stage C: pass 1, wT[4, RPC] = p_pair.T @ I[rows].T ----
        with (
            tc.tile_pool(name="it", bufs=3) as it_pool,
            tc.tile_pool(name="cps", bufs=2, space=bass.MemorySpace.PSUM) as cps_pool,
        ):
            for nh in range(RPC // 512):
                wps = cps_pool.tile([4, 512], dt.float32)
                for mc in range(M // 128):
                    it = it_pool.tile([128, 512], dt.bfloat16)
                    nc.sync.dma_start(
                        it[:],
                        ins["ist"][
                            mc * 128 : (mc + 1) * 128, nh * 512 : (nh + 1) * 512
                        ],
                    )
                    nc.tensor.matmul(
                        wps[:],
                        p_ext[:, mc, :],
                        it[:],
                        start=(mc == 0),
                        stop=(mc == M // 128 - 1),
                    )
                nc.vector.tensor_copy(wT_s[:, nh * 512 : (nh + 1) * 512], wps[:])

        # ---- stage D: assemble w_ext[n, nt, (w_i, w_j, 1)] ----
        with tc.tile_pool(name="dps", bufs=2, space=bass.MemorySpace.PSUM) as dps_pool:
            for nt in range(RPC // 128):
                nsl = slice(nt * 128, (nt + 1) * 128)
                aps = dps_pool.tile([128, 3], dt.float32)
                nc.tensor.matmul(
                    aps[:, 0:3], ones_row[:], d3[:], start=True, stop=False
                )
                nc.tensor.matmul(
                    aps[:, 0:2],
                    wT_s[0:4, nsl],
                    merge4_s[:],
                    start=False,
                    stop=False,
                    skip_group_check=True,
                )
                for x_s, col, vch in (
                    (ixt0_s, 0, 0),
                    (ixt1_s, 0, 1),
                    (jxt0_s, 1, 0),
                    (jxt1_s, 1, 1),
                ):
                    nc.tensor.matmul(
                        aps[:, col : col + 1],
                        x_s[:, nsl],
                        lv[:, vch : vch + 1],
                        start=False,
                        stop=(x_s is jxt1_s),
                        skip_group_check=True,
                    )
                nc.vector.tensor_copy(w_ext[:, nt, :], aps[:])

        # ---- stage E: pass 2, U[3, M] = w_ext.T @ I[rows], blocked for RS ----
        u_d = dram.tile([NCORES, 3, MPC], dt.float32)
        r_d = dram.tile([3, MPC], dt.float32)
        with (
            tc.tile_pool(name="it2", bufs=3) as it2_pool,
            tc.tile_pool(name="eps", bufs=2, space=bass.MemorySpace.PSUM) as eps_pool,
            tc.tile_pool(name="ue", bufs=2) as ue_pool,
        ):
            for mb in range(NCORES):
                for mh in range(MPC // 512):
                    base = mb * MPC + mh * 512
                    ups = eps_pool.tile([3, 512], dt.float32)
                    for nch in range(RPC // 128):
                        it2 = it2_pool.tile([128, 512], dt.bfloat16)
                        nc.sync.dma_start(
                            it2[:],
                            ins["inat"][
                                nch * 128 : (nch + 1) * 128, base : base + 512
                            ],
                        )
                        nc.tensor.matmul(
                            ups[:],
                            w_ext[:, nch, :],
                            it2[:],
                            start=(nch == 0),
                            stop=(nch == RPC // 128 - 1),
                        )
                    ue = ue_pool.tile([3, 512], dt.float32)
                    nc.vector.tensor_copy(ue[:], ups[:])
                    nc.sync.dma_start(
                        u_d[mb, 0:3, mh * 512 : (mh + 1) * 512], ue[:]
                    )

        # ---- stage F: ReduceScatter over hyperlink blocks ----
        nc.gpsimd.collective_compute(
            "ReduceScatter",
            Alu.add,
            replica_groups=[list(range(NCORES))],
            ins=[u_d.opt()],
            outs=[r_d.opt()],
        )

        # ---- stage G: epilogue (each row on its own partition-0 tile:
        # compute engines require partition offsets that are multiples of 32) ----
        with tc.tile_pool(name="epi", bufs=1) as epi_pool:
            q0 = epi_pool.tile([1, MPC], dt.float32)
            q1 = epi_pool.tile([1, MPC], dt.float32)
            den = epi_pool.tile([1, MPC], dt.float32)
            nc.sync.dma_start(q0[:], r_d[0:1, :])
            nc.sync.dma_start(q1[:], r_d[1:2, :])
            nc.sync.dma_start(den[:], r_d[2:3, :])
            rec = epi_pool.tile([1, MPC], dt.float32)
            nc.vector.reciprocal(rec[:], den[:])
            for row, qt in ((0, q0), (1, q1)):
                qr = epi_pool.tile([1, MPC], dt.float32)
                nc.vector.scalar_tensor_tensor(
                    qr[:], qt[:], 1.0, rec[:], op0=Alu.mult, op1=Alu.mult
                )
                s_s = epi_pool.tile([1, MPC], dt.float32)
                nc.scalar.activation(
                    s_s[:], qr[:], Act.Sigmoid, bias=intb2_s[0:1, :]
                )
                nc.sync.dma_start(out_ap[row : row + 1, :], s_s[:])


def _build():
    global _NC
    if _NC is not None:
        return _NC
    nc = bacc.Bacc(
        "TRN2",
        target_bir_lowering=False,
        debug=False,
        enable_asserts=True,
        num_devices=NCORES,
    )
    dt = mybir.dt
    specs = [
        ("ist", [M, RPC], dt.bfloat16),
        ("inat", [RPC, M], dt.bfloat16),
        ("iaxt", [D, M], dt.float32),
        ("jaxt", [D, M], dt.float32),
        ("ixt", [D, RPC], dt.bfloat16),
        ("jxt", [D, RPC], dt.bfloat16),
        ("gcnwt", [H, D], dt.float32),
        ("loopwt", [H, D], dt.float32),
        ("intw", [H, 1], dt.float32),
        ("gcnb2", [H, 2], dt.float32),
        ("loopb2", [H, 2], dt.float32),
        ("intb2", [2, 1], dt.float32),
        ("merge4", [4, 2], dt.bfloat16),
    ]
    ins = {
        name: nc.dram_tensor(name, shape, d, kind="ExternalInput").ap()
        for name, shape, d in specs
    }
    out_ap = nc.dram_tensor("out", [2, MPC], dt.float32, kind="ExternalOutput").ap()
    with tile.TileContext(nc) as tc:
        _emit(nc, tc, ins, out_ap)
    nc.compile()
    _NC = nc
    return nc


def _prepare_in_maps(iX, jX, iAX, jAX, I, loop_w, loop_b, gcn_w, gcn_b, int_w, int_b):
    iaxt = np.ascontiguousarray(iAX.T, dtype=np.float32)
    jaxt = np.ascontiguousarray(jAX.T, dtype=np.float32)
    gcnwt = np.ascontiguousarray(gcn_w.T, dtype=np.float32)
    loopwt = np.ascontiguousarray(loop_w.T, dtype=np.float32)
    intw = np.asarray(int_w, np.float32)
    gcnb2 = np.repeat(np.asarray(gcn_b, np.float32).reshape(H, 1), 2, axis=1)
    loopb2 = np.repeat(np.asarray(loop_b, np.float32).reshape(H, 1), 2, axis=1)
    intb2 = np.full((2, 1), np.float32(np.asarray(int_b).reshape(-1)[0]), dtype=np.float32)
    merge4 = np.array([[1, 0], [0, 1], [1, 0], [0, 1]], dtype=BF16)

    in_maps = []
    for k in range(NCORES):
        rows = slice(k * RPC, (k + 1) * RPC)
        in_maps.append(
            {
                "ist": I[rows].T.astype(BF16),
                "inat": I[rows].astype(BF16),
                "iaxt": iaxt,
                "jaxt": jaxt,
                "ixt": iX[rows].T.astype(BF16),
                "jxt": jX[rows].T.astype(BF16),
                "gcnwt": gcnwt,
                "loopwt": loopwt,
                "intw": intw,
                "gcnb2": gcnb2,
                "loopb2": loopb2,
                "intb2": intb2,
                "merge4": merge4,
            }
        )
    return in_maps


def kernel(**inputs):
    global LAST_EXEC_NS, LAST_TRACE_DIR
    nc = _build()
    in_maps = _prepare_in_maps(**inputs)

    res = run_bass_kernel_spmd(
        nc, in_maps, core_ids=list(range(NCORES)), trace=PROFILE
    )
    LAST_EXEC_NS = getattr(res, "exec_time_ns", None)
    LAST_TRACE_DIR = getattr(res, "tmpdir", None)

    S = np.empty((M, 1), dtype=np.float32)
    S_ = np.empty((M, 1), dtype=np.float32)
    for k in range(NCORES):
        o = np.asarray(res.results[k]["out"], dtype=np.float32)
        S[k * MPC : (k + 1) * MPC, 0] = o[0]
        S_[k * MPC : (k + 1) * MPC, 0] = o[1]
    return (S, S_)


# revision 16
# speedup vs baseline: 1.1785x; 1.0987x over previous
import os
import sys

if "/opt/trn_rl_repo" not in sys.path:
    sys.path.insert(0, "/opt/trn_rl_repo")

import numpy as np
import ml_dtypes

import concourse.bass as bass
import concourse.tile as tile
from concourse import bacc
from concourse.bass_utils import run_bass_kernel_spmd

mybir = bass.mybir

N = 8192
M = 8192
D = 256
H = 128
NCORES = 8
RPC = N // NCORES  # 1024 rows per core
MPC = M // NCORES  # 1024 hyperlink cols per core (RS split)

BF16 = ml_dtypes.bfloat16

PROFILE = False
LAST_EXEC_NS = None
LAST_TRACE_DIR = None

_NC = None


def _emit(nc, tc, ins, out_ap):
    dt = mybir.dt
    Alu = mybir.AluOpType
    Act = mybir.ActivationFunctionType

    with (
        tc.tile_pool(name="persist", bufs=1) as pp,
        tc.tile_pool(name="dram", bufs=1, space="DRAM") as dram,
    ):
        # ---- persistent SBUF tensors ----
        # The p-chain (c, vg, iAX products) must be f32: q = I.T@(I@p)/colsum
        # amplifies any bias in mean(p) by ~4096x, so bf16 rounding there
        # shifts the logits by O(0.3). p is then split into a bf16 hi+lo
        # pair so the big pass-1 matmul can stay bf16.
        gcnwt_s = pp.tile([H, D], dt.float32)
        loopwt_s = pp.tile([H, D], dt.float32)
        intw_s = pp.tile([H, 1], dt.float32)
        gcnb2_s = pp.tile([H, 2], dt.float32)
        loopb2_s = pp.tile([H, 2], dt.float32)
        intb2_s = pp.tile([2, 1], dt.float32)
        ixt0_s = pp.tile([128, RPC], dt.bfloat16)
        ixt1_s = pp.tile([128, RPC], dt.bfloat16)
        jxt0_s = pp.tile([128, RPC], dt.bfloat16)
        jxt1_s = pp.tile([128, RPC], dt.bfloat16)
        ones_row = pp.tile([1, 128], dt.bfloat16)
        ones_f32 = pp.tile([1, 128], dt.float32)
        merge4_s = pp.tile([4, 2], dt.bfloat16)
        vg = pp.tile([128, 2], dt.float32)
        lv = pp.tile([128, 2], dt.bfloat16)
        c2 = pp.tile([1, 2], dt.float32)
        d3 = pp.tile([1, 3], dt.bfloat16)
        p_ext = pp.tile([128, M // 128, 4], dt.bfloat16)
        wT_s = pp.tile([4, RPC], dt.bfloat16)
        w_ext = pp.tile([128, RPC // 128, 3], dt.bfloat16)

        nc.sync.dma_start(gcnwt_s[:], ins["gcnwt"][:])
        nc.sync.dma_start(loopwt_s[:], ins["loopwt"][:])
        nc.sync.dma_start(intw_s[:], ins["intw"][:])
        nc.sync.dma_start(gcnb2_s[:], ins["gcnb2"][:])
        nc.sync.dma_start(loopb2_s[:], ins["loopb2"][:])
        nc.sync.dma_start(intb2_s[:], ins["intb2"][:])
        nc.sync.dma_start(merge4_s[:], ins["merge4"][:])
        nc.sync.dma_start(ixt0_s[:], ins["ixt"][0:128, :])
        nc.sync.dma_start(ixt1_s[:], ins["ixt"][128:256, :])
        nc.sync.dma_start(jxt0_s[:], ins["jxt"][0:128, :])
        nc.sync.dma_start(jxt1_s[:], ins["jxt"][128:256, :])
        nc.vector.memset(ones_row[:], 1.0)
        nc.vector.memset(ones_f32[:], 1.0)

        # ---- stage A: fold int_w through weights (all f32) ----
        with tc.tile_pool(name="aps", bufs=2, space=bass.MemorySpace.PSUM) as aps_pool:
            for wt_s, dst in ((gcnwt_s, vg), (loopwt_s, lv)):
                ps = aps_pool.tile([128, 2], dt.float32)
                for ch in range(2):
                    nc.tensor.matmul(
                        ps[:, ch : ch + 1],
                        wt_s[:, ch * 128 : (ch + 1) * 128],
                        intw_s[:],
                        start=True,
                        stop=True,
                    )
                nc.vector.tensor_copy(dst[:], ps[:])
            for b_s, dst in ((gcnb2_s, c2), (loopb2_s, d3)):
                ps = aps_pool.tile([1, 2], dt.float32)
                nc.tensor.matmul(ps[:], intw_s[:], b_s[:], start=True, stop=True)
                nc.vector.tensor_copy(dst[:, 0:2], ps[:])
        nc.vector.memset(d3[:, 2:3], 1.0)

        # ---- stage B: p = AX @ v + c in f32, split to bf16 hi/lo pair,
        # laid out [128, 64, (i_hi, j_hi, i_lo, j_lo)] (m on partitions) ----
        with (
            tc.tile_pool(name="ax", bufs=2) as ax_pool,
            tc.tile_pool(name="spl", bufs=2) as spl_pool,
            tc.tile_pool(name="bps", bufs=2, space=bass.MemorySpace.PSUM) as bps_pool,
        ):
            for mg in range(M // 512):
                axi0 = ax_pool.tile([128, 512], dt.float32)
                axi1 = ax_pool.tile([128, 512], dt.float32)
                axj0 = ax_pool.tile([128, 512], dt.float32)
                axj1 = ax_pool.tile([128, 512], dt.float32)
                sl = slice(mg * 512, (mg + 1) * 512)
                nc.sync.dma_start(axi0[:], ins["iaxt"][0:128, sl])
                nc.sync.dma_start(axi1[:], ins["iaxt"][128:256, sl])
                nc.sync.dma_start(axj0[:], ins["jaxt"][0:128, sl])
                nc.sync.dma_start(axj1[:], ins["jaxt"][128:256, sl])
                for mt in range(4):
                    mc = mg * 4 + mt
                    msl = slice(mt * 128, (mt + 1) * 128)
                    psp = bps_pool.tile([128, 2], dt.float32)
                    nc.tensor.matmul(
                        psp[:, 0:2], ones_f32[:], c2[:], start=True, stop=False
                    )
                    for src, col in ((axi0, 0), (axi1, 0), (axj0, 1), (axj1, 1)):
                        vch = 0 if src in (axi0, axj0) else 1
                        nc.tensor.matmul(
                            psp[:, col : col + 1],
                            src[:, msl],
                            vg[:, vch : vch + 1],
                            start=False,
                            stop=(src is axj1),
                            skip_group_check=True,
                        )
                    nc.vector.tensor_copy(p_ext[:, mc, 0:2], psp[:])
                    hi_f32 = spl_pool.tile([128, 2], dt.float32)
                    nc.vector.tensor_copy(hi_f32[:], p_ext[:, mc, 0:2])
                    nc.vector.scalar_tensor_tensor(
                        p_ext[:, mc, 2:4],
                        psp[:],
                        1.0,
                        hi_f32[:],
                        op0=Alu.mult,
                        op1=Alu.subtract,
                    )

        # ---- stage C: pass 1, wT[4, RPC] = p_pair.T @ I[rows].T ----
        with (
            tc.tile_pool(name="it", bufs=3) as it_pool,
            tc.tile_pool(name="cps", bufs=2, space=bass.MemorySpace.PSUM) as cps_pool,
        ):
            for nh in range(RPC // 512):
                wps = cps_pool.tile([4, 512], dt.float32)
                for mc in range(M // 128):
                    it = it_pool.tile([128, 512], dt.bfloat16)
                    nc.sync.dma_start(
                        it[:],
                        ins["ist"][
                            mc * 128 : (mc + 1) * 128, nh * 512 : (nh + 1) * 512
                        ],
                    )
                    nc.tensor.matmul(
                        wps[:],
                        p_ext[:, mc, :],
                        it[:],
                        start=(mc == 0),
                        stop=(mc == M // 128 - 1),
                    )
                nc.vector.tensor_copy(wT_s[:, nh * 512 : (nh + 1) * 512], wps[:])

        # ---- stage D: assemble w_ext[n, nt, (w_i, w_j, 1)] ----
        with tc.tile_pool(name="dps", bufs=2, space=bass.MemorySpace.PSUM) as dps_pool:
            for nt in range(RPC // 128):
                nsl = slice(nt * 128, (nt + 1) * 128)
                aps = dps_pool.tile([128, 3], dt.float32)
                nc.tensor.matmul(
                    aps[:, 0:3], ones_row[:], d3[:], start=True, stop=False
                )
                nc.tensor.matmul(
                    aps[:, 0:2],
                    wT_s[0:4, nsl],
                    merge4_s[:],
                    start=False,
                    stop=False,
                    skip_group_check=True,
                )
                for x_s, col, vch in (
                    (ixt0_s, 0, 0),
                    (ixt1_s, 0, 1),
                    (jxt0_s, 1, 0),
                    (jxt1_s, 1, 1),
                ):
                    nc.tensor.matmul(
                        aps[:, col : col + 1],
                        x_s[:, nsl],
                        lv[:, vch : vch + 1],
                        start=False,
                        stop=(x_s is jxt1_s),
                        skip_group_check=True,
                    )
                nc.vector.tensor_copy(w_ext[:, nt, :], aps[:])

        # ---- stage E: pass 2, U[3, M] = w_ext.T @ I[rows], blocked for RS ----
        u_d = dram.tile([NCORES, 3, MPC], dt.float32)
        r_d = dram.tile([3, MPC], dt.float32)
        with (
            tc.tile_pool(name="it2", bufs=3) as it2_pool,
            tc.tile_pool(name="eps", bufs=2, space=bass.MemorySpace.PSUM) as eps_pool,
            tc.tile_pool(name="ue", bufs=2) as ue_pool,
        ):
            for mb in range(NCORES):
                for mh in range(MPC // 512):
                    base = mb * MPC + mh * 512
                    ups = eps_pool.tile([3, 512], dt.float32)
                    for nch in range(RPC // 128):
                        it2 = it2_pool.tile([128, 512], dt.bfloat16)
                        nc.sync.dma_start(
                            it2[:],
                            ins["inat"][
                                nch * 128 : (nch + 1) * 128, base : base + 512
                            ],
                        )
                        nc.tensor.matmul(
                            ups[:],
                            w_ext[:, nch, :],
                            it2[:],
                            start=(nch == 0),
                            stop=(nch == RPC // 128 - 1),
                        )
                    ue = ue_pool.tile([3, 512], dt.float32)
                    nc.vector.tensor_copy(ue[:], ups[:])
                    nc.sync.dma_start(
                        u_d[mb, 0:3, mh * 512 : (mh + 1) * 512], ue[:]
                    )

        # ---- stage F: ReduceScatter over hyperlink blocks ----
        nc.gpsimd.collective_compute(
            "ReduceScatter",
            Alu.add,
            replica_groups=[list(range(NCORES))],
            ins=[u_d.opt()],
            outs=[r_d.opt()],
        )

        # ---- stage G: epilogue (each row on its own partition-0 tile:
        # compute engines require partition offsets that are multiples of 32) ----
        with tc.tile_pool(name="epi", bufs=1) as epi_pool:
            q0 = epi_pool.tile([1, MPC], dt.float32)
            q1 = epi_pool.tile([1, MPC], dt.float32)
            den = epi_pool.tile([1, MPC], dt.float32)
            nc.sync.dma_start(q0[:], r_d[0:1, :])
            nc.sync.dma_start(q1[:], r_d[1:2, :])
            nc.sync.dma_start(den[:], r_d[2:3, :])
            rec = epi_pool.tile([1, MPC], dt.float32)
            nc.vector.reciprocal(rec[:], den[:])
            for row, qt in ((0, q0), (1, q1)):
                qr = epi_pool.tile([1, MPC], dt.float32)
                nc.vector.scalar_tensor_tensor(
                    qr[:], qt[:], 1.0, rec[:], op0=Alu.mult, op1=Alu.mult
                )
                s_s = epi_pool.tile([1, MPC], dt.float32)
                nc.scalar.activation(
                    s_s[:], qr[:], Act.Sigmoid, bias=intb2_s[0:1, :]
                )
                nc.sync.dma_start(out_ap[row : row + 1, :], s_s[:])


def _build():
    global _NC
    if _NC is not None:
        return _NC
    reps = int(os.environ.get("KREPS", "1"))
    nc = bacc.Bacc(
        "TRN2",
        target_bir_lowering=False,
        debug=False,
        enable_asserts=True,
        num_devices=NCORES,
    )
    dt = mybir.dt
    specs = [
        ("ist", [M, RPC], dt.bfloat16),
        ("inat", [RPC, M], dt.bfloat16),
        ("iaxt", [D, M], dt.float32),
        ("jaxt", [D, M], dt.float32),
        ("ixt", [D, RPC], dt.bfloat16),
        ("jxt", [D, RPC], dt.bfloat16),
        ("gcnwt", [H, D], dt.float32),
        ("loopwt", [H, D], dt.float32),
        ("intw", [H, 1], dt.float32),
        ("gcnb2", [H, 2], dt.float32),
        ("loopb2", [H, 2], dt.float32),
        ("intb2", [2, 1], dt.float32),
        ("merge4", [4, 2], dt.bfloat16),
    ]
    ins = {
        name: nc.dram_tensor(name, shape, d, kind="ExternalInput").ap()
        for name, shape, d in specs
    }
    out_ap = nc.dram_tensor("out", [2, MPC], dt.float32, kind="ExternalOutput").ap()
    with tile.TileContext(nc) as tc:
        for _ in range(reps):
            _emit(nc, tc, ins, out_ap)
    nc.compile()
    _NC = nc
    return nc


def _prepare_in_maps(iX, jX, iAX, jAX, I, loop_w, loop_b, gcn_w, gcn_b, int_w, int_b):
    iaxt = np.ascontiguousarray(iAX.T, dtype=np.float32)
    jaxt = np.ascontiguousarray(jAX.T, dtype=np.float32)
    gcnwt = np.ascontiguousarray(gcn_w.T, dtype=np.float32)
    loopwt = np.ascontiguousarray(loop_w.T, dtype=np.float32)
    intw = np.asarray(int_w, np.float32)
    gcnb2 = np.repeat(np.asarray(gcn_b, np.float32).reshape(H, 1), 2, axis=1)
    loopb2 = np.repeat(np.asarray(loop_b, np.float32).reshape(H, 1), 2, axis=1)
    intb2 = np.full((2, 1), np.float32(np.asarray(int_b).reshape(-1)[0]), dtype=np.float32)
    merge4 = np.array([[1, 0], [0, 1], [1, 0], [0, 1]], dtype=BF16)

    in_maps = []
    for k in range(NCORES):
        rows = slice(k * RPC, (k + 1) * RPC)
        in_maps.append(
            {
                "ist": I[rows].T.astype(BF16),
                "inat": I[rows].astype(BF16),
                "iaxt": iaxt,
                "jaxt": jaxt,
                "ixt": iX[rows].T.astype(BF16),
                "jxt": jX[rows].T.astype(BF16),
                "gcnwt": gcnwt,
                "loopwt": loopwt,
                "intw": intw,
                "gcnb2": gcnb2,
                "loopb2": loopb2,
                "intb2": intb2,
                "merge4": merge4,
            }
        )
    return in_maps


def kernel(**inputs):
    global LAST_EXEC_NS, LAST_TRACE_DIR
    nc = _build()
    in_maps = _prepare_in_maps(**inputs)

    res = run_bass_kernel_spmd(
        nc, in_maps, core_ids=list(range(NCORES)), trace=PROFILE
    )
    LAST_EXEC_NS = getattr(res, "exec_time_ns", None)
    LAST_TRACE_DIR = getattr(res, "tmpdir", None)

    S = np.empty((M, 1), dtype=np.float32)
    S_ = np.empty((M, 1), dtype=np.float32)
    for k in range(NCORES):
        o = np.asarray(res.results[k]["out"], dtype=np.float32)
        S[k * MPC : (k + 1) * MPC, 0] = o[0]
        S_[k * MPC : (k + 1) * MPC, 0] = o[1]
    return (S, S_)
